# revision 7
# baseline (speedup 1.0000x reference)
"""DGCNN edge-conv block on 8 Trainium2 NeuronCores.

Sharding: data-parallel over (batch, query-half): core i handles batch i//2,
queries [2048*(i%2) : +2048] of that batch's 4096 points. Each core gets the
full point cloud of its batch (keys) with columns permuted so its own queries
are always columns 0..2047 (SPMD: one program, per-core inputs).

Numerics: x is fp16 everywhere (distances AND conv1 inputs). KNN scores
s/2 = x16_q.x16_k - xx16(k)/2 computed in fp32 PSUM = the exact knn of the
fp16-quantized points; the max-over-k structure makes neighbor ORDER
irrelevant - only the set matters. The -xx/2 per-key offset is computed on
HOST (aux stats, like the BN folding) and rides in spare contraction rows
1000-1002 of a host-built key-side chunk-7 copy (xk7): query side carries
1.0 there, xk7 rows 104-106 carry a 3-way fp16 split of -xx/2, so the whole
score is ONE 8-matmul PSUM group per (query-tile, key-tile).
Conv weights/activations fp16, fp32 PSUM, BN+ReLU writes fp32.

Pipeline per core (conv segments interleave into the knn loop):
  A: load x + xk7 quarter-major; A=w1n@x (duplicated to partitions 64-127
     so the neighbor gather can use all 8 gpsimd cores), Bv=w1c@x_q
  B: per query-tile: 8 key-tiles x 8 fp16 matmuls; top-8 via DVE
     max/max_index; top-3 indices -> ap_gather of A columns -> h1pre;
     h1 = relu(bn1(h1pre+Bv)) applied per query-tile
  C (per query segment, emitted between knn blocks; final segments are
     1 query-tile wide to shrink the tail): conv2..conv4 with max-over-k
     (conv4's BN+ReLU deferred past the max: BN scale > 0), cat,
     conv5 -> out [1024, 2048] fp32
"""

import sys

sys.path.insert(0, "/opt/trn_rl_repo")

import numpy as np

B, C_IN, N, K = 4, 1000, 4096, 3
CPAD = 1024        # padded contraction dim
NQ = 2048          # queries per core
CP = 128           # contraction chunk partitions
CH = 8             # number of contraction chunks
NT = 512           # key tile (psum bank width in fp32)
NNT = N // NT      # 8 key tiles
QT = 128           # query tile (psum partitions)
NQT = NQ // QT     # 16 query tiles
NQH = 1024         # quarter width (phase A / DMA granularity)
EPS = np.float32(1e-5)

_CACHE = {}


def build_nc(finalize=True):
    import concourse.mybir as mybir
    import concourse.tile as tile
    from concourse import bacc

    f32 = mybir.dt.float32
    f16 = mybir.dt.float16
    u16 = mybir.dt.uint16
    i16 = mybir.dt.int16
    Relu = mybir.ActivationFunctionType.Relu

    nc = bacc.Bacc("TRN2", target_bir_lowering=False, debug=False, num_devices=8)

    xh = nc.dram_tensor("xh", [CPAD, N], f16, kind="ExternalInput").ap()
    xk7d = nc.dram_tensor("xk7d", [CP, N], f16, kind="ExternalInput").ap()
    w1t = nc.dram_tensor("w1t", [CPAD, 128], f16, kind="ExternalInput").ap()
    w2t = nc.dram_tensor("w2t", [64, 128], f16, kind="ExternalInput").ap()
    w3t = nc.dram_tensor("w3t", [128, 256], f16, kind="ExternalInput").ap()
    w4t = nc.dram_tensor("w4t", [256, 512], f16, kind="ExternalInput").ap()
    w5p = nc.dram_tensor("w5p", [128, 8, 1024], f16, kind="ExternalInput").ap()
    sb1 = nc.dram_tensor("sb1", [64, 2], f32, kind="ExternalInput").ap()
    sb2 = nc.dram_tensor("sb2", [128, 2], f32, kind="ExternalInput").ap()
    sb3 = nc.dram_tensor("sb3", [128, 4], f32, kind="ExternalInput").ap()
    sb4 = nc.dram_tensor("sb4", [128, 8], f32, kind="ExternalInput").ap()
    sb5 = nc.dram_tensor("sb5", [128, 16], f32, kind="ExternalInput").ap()
    out = nc.dram_tensor("out", [1024, NQ], f32, kind="ExternalOutput").ap()

    with tile.TileContext(nc) as tc:
        _body(nc, tc, mybir, xh, xk7d, w1t, w2t, w3t, w4t, w5p,
              sb1, sb2, sb3, sb4, sb5, out, f32, f16, u16, i16, Relu)
    if finalize:
        nc.finalize()
    return nc


def _body(nc, tc, mybir, xh, xk7d, w1t, w2t, w3t, w4t, w5p,
          sb1, sb2, sb3, sb4, sb5, out, f32, f16, u16, i16, Relu):
    from contextlib import ExitStack
    from concourse import library_config

    es = ExitStack()
    with es:
        p_c1 = es.enter_context(tc.tile_pool(name="c1", bufs=1))

        # gpsimd library for the gathers; dummy gather + drain force the
        # ucode load now so it overlaps the early phases.
        nc.gpsimd.load_library(library_config.ap_gather)
        dmy = p_c1.tile([64, 16], f32, tag="dmy")
        dmys = p_c1.tile([64, 4], f32, tag="dmys")
        dmyi = p_c1.tile([64, 1], i16, tag="dmyi")
        nc.vector.memset(dmys[:], 0.0)
        nc.vector.memset(dmyi[:], 0)
        nc.gpsimd.ap_gather(out_ap=dmy[:], in_ap=dmys[:], idxs_ap=dmyi[:],
                            channels=64, num_elems=4, d=1, num_idxs=16)
        nc.gpsimd.drain()

        # ---- persistent tiles ----
        w1s = p_c1.tile([CP, CH, 128], f16, tag="w1s")
        nc.sync.dma_start(w1s[:], w1t.rearrange("(c p) m -> p c m", p=CP))
        # h1 pre-activation, fp16, kk-major q-ordered [64, 3*NQ]
        h1pre = p_c1.tile([64, 3 * NQ], f16, tag="h1pre")
        # A duplicated on partitions 64-127 so ap_gather uses all 8 cores
        A = p_c1.tile([128, N], f32, tag="A")
        Bv = p_c1.tile([64, NQ], f32, tag="Bv")
        # key-side copy of contraction chunk 7 (built on host): data rows
        # 0-103, rows 104-106 carry the 3-way fp16 split of -xx/2 (query
        # side has 1.0 there), rows 107-127 zero.
        xk7 = p_c1.tile([CP, N], f16, tag="xk7")
        xhs = p_c1.tile([CP, CH, N], f16, tag="xhs")
        xhr = xh.rearrange("(c p) n -> p c n", p=CP)
        # quarter-major loads so phase A / knn can start on quarter 0
        for q4 in range(N // NQH):
            qsl = slice(q4 * NQH, (q4 + 1) * NQH)
            for c in range(CH):
                nc.sync.dma_start(xhs[:, c, qsl], xhr[:, c, qsl])
            nc.sync.dma_start(xk7[:, qsl], xk7d[:, qsl])

        # conv weights + folded BN scale/bias (needed from seg 0 on)
        w2s = p_c1.tile([64, 128], f16, tag="w2s")
        nc.sync.dma_start(w2s[:], w2t[:])
        w3s = p_c1.tile([128, 256], f16, tag="w3s")
        nc.sync.dma_start(w3s[:], w3t[:])
        w4s = p_c1.tile([128, 2, 512], f16, tag="w4s")
        nc.sync.dma_start(w4s[:], w4t.rearrange("(c p) m -> p c m", p=128))
        w5s = p_c1.tile([128, 8, 1024], f16, tag="w5s")
        nc.sync.dma_start(w5s[:], w5p[:])
        sb1s = p_c1.tile([64, 2], f32, tag="sb1s")
        nc.sync.dma_start(sb1s[:], sb1[:])
        sb2s = p_c1.tile([128, 2], f32, tag="sb2s")
        nc.sync.dma_start(sb2s[:], sb2[:])
        sb3s = p_c1.tile([128, 4], f32, tag="sb3s")
        nc.sync.dma_start(sb3s[:], sb3[:])
        sb4s = p_c1.tile([128, 8], f32, tag="sb4s")
        nc.sync.dma_start(sb4s[:], sb4[:])
        sb5s = p_c1.tile([128, 16], f32, tag="sb5s")
        nc.sync.dma_start(sb5s[:], sb5[:])

        # ---- phase A: A/Bv in 4 quarter-passes (xx comes from host) ----
        with nc.named_scope("prep"):
            with tc.tile_pool(name="psa", bufs=2, space="PSUM") as p_psa:
                for q4 in range(N // NQH):
                    nts = [q4 * 2, q4 * 2 + 1]
                    pav = [p_psa.tile([128, NT], f32, tag="pa",
                                      name=f"pa{q4}_{_j}")
                           for _j in range(2)]
                    for c in range(CH):
                        for j, nt in enumerate(nts):
                            mw = 128 if nt < NQ // NT else 64
                            nc.tensor.matmul(
                                pav[j][0:mw, :], w1s[:, c, 0:mw],
                                xhs[:, c, nt * NT:(nt + 1) * NT],
                                start=(c == 0), stop=(c == CH - 1))
                    for j, nt in enumerate(nts):
                        ns = slice(nt * NT, (nt + 1) * NT)
                        nc.scalar.copy(A[0:64, ns], pav[j][0:64, :])
                        nc.scalar.copy(A[64:128, ns], pav[j][0:64, :])
                        if nt < NQ // NT:
                            nc.scalar.copy(Bv[:, ns], pav[j][64:128, :])

        # ---- phase B (knn) with conv segments interleaved ----
        with tc.tile_pool(name="ms", bufs=1) as p_s, \
             tc.tile_pool(name="m8", bufs=3) as p_m8, \
             tc.tile_pool(name="ixw", bufs=4) as p_ixw, \
             tc.tile_pool(name="wtd", bufs=3, space="DRAM") as p_wtd, \
             tc.tile_pool(name="gq", bufs=3) as p_gq, \
             tc.tile_pool(name="seg", bufs=1) as p_seg, \
             tc.tile_pool(name="sg2", bufs=2) as p_sg2, \
             tc.tile_pool(name="tmp", bufs=2) as p_tmp, \
             tc.tile_pool(name="osb", bufs=2) as p_osb, \
             tc.tile_pool(name="pss", bufs=4, space="PSUM") as p_pss, \
             tc.tile_pool(name="psd", bufs=4, space="PSUM") as p_psd:
            outr = out.rearrange("(c p) n -> p c n", p=128)
            h13 = h1pre.rearrange("p (k q) -> p k q", k=3)

            srows = {}

            def knn_mm(qt, nts):
                qs = slice(qt * QT, (qt + 1) * QT)
                if qt not in srows:
                    srows[qt] = p_s.tile([QT, N], f32, tag="srow", bufs=3,
                                         name=f"srow{qt}")
                srow = srows[qt]
                for nt in nts:
                    ns = slice(nt * NT, (nt + 1) * NT)
                    ps = p_pss.tile([QT, NT], f32, tag="pss",
                                    name=f"pss{qt}_{nt}")
                    for c in range(CH - 1):
                        nc.tensor.matmul(ps[:], xhs[:, c, qs],
                                         xhs[:, c, ns],
                                         start=(c == 0), stop=False)
                    nc.tensor.matmul(ps[:], xhs[:, CH - 1, qs], xk7[:, ns],
                                     start=False, stop=True)
                    # drain PSUM on alternating engines so neither the
                    # scalar ACT chain nor the DVE top-8 chain bottlenecks
                    # the pss bank recycle
                    if nt % 2 == 0:
                        nc.scalar.copy(srow[:, ns], ps[:])
                    else:
                        nc.vector.tensor_copy(srow[:, ns], ps[:])

            def knn_top(qt):
                srow = srows[qt]
                m8 = p_m8.tile([QT, 8], f32, tag="m8")
                i8 = p_m8.tile([QT, 8], u16, tag="i8")
                nc.vector.max(out=m8[:], in_=srow[:])
                nc.vector.max_index(out=i8[:], in_max=m8[:],
                                    in_values=srow[:])
                # wrap top-3 indices into ap_gather layout, one list per
                # query half-tile (gpsimd cores 0-3 serve queries 0-63,
                # cores 4-7 queries 64-127):
                # idxw[16g''+r, 3g+kk] = i8[64h + 16g+r, kk], g''=0..3.
                # i8 bounces through DRAM (where the partition regroup is
                # plain address math) and fans back out on parallel queues.
                idxw = p_ixw.tile([128, 12], i16, tag="idxw")
                wt = p_wtd.tile([128, 3], i16, tag="wt")
                nc.sync.dma_start(wt[:], i8[:, 0:3].bitcast(i16))
                for h in range(2):
                    wr = wt[64 * h:64 * h + 64, :] \
                        .rearrange("(g r) k -> r g k", g=4)
                    for gpp in range(4):
                        p0 = 64 * h + 16 * gpp
                        nc.sync.dma_start(
                            idxw[p0:p0 + 16, :]
                            .rearrange("p (g k) -> p g k", g=4), wr)
                # gather this qt's neighbor features (gpsimd, overlaps PE);
                # within each half, gather position 16*(g*3+kk)+r is
                # (query 16g+r, neighbor kk)
                gq = p_gq.tile([128, 3 * 64], f32, tag="gq")
                nc.gpsimd.ap_gather(
                    out_ap=gq[:], in_ap=A[:], idxs_ap=idxw[:],
                    channels=128, num_elems=N, d=1, num_idxs=3 * 64)
                # unpermute into h1pre (fp16, kk-major q-order)
                gqv = gq.rearrange("p (g kk r) -> p g kk r",
                                   g=4, kk=3, r=16)
                for h in range(2):
                    q0 = qt * QT + 64 * h
                    dst = h13[:, :, q0:q0 + 64] \
                        .rearrange("p kk (g r) -> p g kk r", g=4)
                    nc.scalar.copy(dst, gqv[64 * h:64 * h + 64])
                # h1 = relu(bn1(gathered + center)) in place, fp16
                qs = slice(qt * QT, (qt + 1) * QT)
                hseg = h13[:, :, qs]
                bvb = Bv[:, qs].unsqueeze(1).to_broadcast([64, 3, QT])
                nc.vector.tensor_add(hseg, hseg, bvb)
                nc.scalar.activation(hseg, hseg, Relu,
                                     bias=sb1s[:, 1:2], scale=sb1s[:, 0:1])

            def knn_qt(qt):
                knn_mm(qt, range(NNT))
                knn_top(qt)

            def conv_seg(qt0, qt1):
                qs = slice(qt0 * QT, qt1 * QT)
                W = (qt1 - qt0) * QT
                h2 = p_sg2.tile([128, 3, W], f16, tag="h2", bufs=1,
                                name=f"h2_{qt0}")
                h3 = p_seg.tile([128, 2, 3, W], f16, tag="h3",
                                name=f"h3_{qt0}")
                cat = p_seg.tile([128, 8, W], f16, tag="cat",
                                 name=f"cat_{qt0}")
                nc.vector.memset(cat[64:128, 0, :], 0.0)

                # conv2 (K=64 -> 128)
                for kk in range(3):
                    ps2 = p_psd.tile([128, W], f32, tag="psd",
                                     name=f"ps2_{qt0}_{kk}")
                    nc.tensor.matmul(ps2[:], w2s[:], h13[:, kk, qs],
                                     start=True, stop=True)
                    nc.scalar.activation(h2[:, kk, :], ps2[:], Relu,
                                         bias=sb2s[:, 1:2], scale=sb2s[:, 0:1])
                # x1 -> cat chunk 0 (64 rows)
                t1 = p_tmp.tile([64, W], f16, tag="t64", bufs=1,
                                name=f"t1_{qt0}")
                nc.vector.tensor_max(t1[:], h13[:, 1, qs], h13[:, 2, qs])
                nc.vector.tensor_max(cat[0:64, 0, :], t1[:], h13[:, 0, qs])
                # x2 -> cat chunk 1
                t2 = p_tmp.tile([128, W], f16, tag="t128", name=f"t2_{qt0}")
                nc.vector.tensor_max(t2[:], h2[:, 1, :], h2[:, 2, :])
                nc.vector.tensor_max(cat[:, 1, :], t2[:], h2[:, 0, :])

                # conv3 (K=128 -> 256 in 2 chunks)
                for m in range(2):
                    for kk in range(3):
                        ps3 = p_psd.tile([128, W], f32, tag="psd",
                                         name=f"ps3_{qt0}_{m}_{kk}")
                        nc.tensor.matmul(ps3[:], w3s[:, m * 128:(m + 1) * 128],
                                         h2[:, kk, :], start=True, stop=True)
                        nc.scalar.activation(h3[:, m, kk, :], ps3[:], Relu,
                                             bias=sb3s[:, 2 + m:3 + m],
                                             scale=sb3s[:, m:m + 1])
                # x3 -> cat chunks 2,3
                for m in range(2):
                    t3 = p_tmp.tile([128, W], f16, tag="t128",
                                    name=f"t3_{qt0}_{m}")
                    nc.vector.tensor_max(t3[:], h3[:, m, 1, :], h3[:, m, 2, :])
                    nc.vector.tensor_max(cat[:, 2 + m, :], t3[:], h3[:, m, 0, :])

                # conv4 (K=256 in 2 chunks -> 512 in 4 chunks); BN+ReLU
                # deferred past the max over kk (BN scale > 0)
                for m in range(4):
                    ps4 = [p_psd.tile([128, W], f32, tag="psd",
                                      name=f"ps4_{qt0}_{m}_{kk}")
                           for kk in range(3)]
                    for kk in range(3):
                        for c in range(2):
                            nc.tensor.matmul(
                                ps4[kk][:], w4s[:, c, m * 128:(m + 1) * 128],
                                h3[:, c, kk, :], start=(c == 0), stop=(c == 1))
                    t4 = p_tmp.tile([128, W], f32, tag="t128f", bufs=1,
                                    name=f"t4_{qt0}_{m}")
                    nc.scalar.copy(t4[:], ps4[0][:])
                    nc.vector.tensor_max(t4[:], t4[:], ps4[1][:])
                    nc.vector.tensor_max(t4[:], t4[:], ps4[2][:])
                    nc.scalar.activation(cat[:, 4 + m, :], t4[:], Relu,
                                         bias=sb4s[:, 4 + m:5 + m],
                                         scale=sb4s[:, m:m + 1])

                # conv5 (K=960 padded to 8*128 -> 1024 in 8 chunks)
                for m in range(8):
                    ps5 = p_psd.tile([128, W], f32, tag="psd",
                                     name=f"ps5_{qt0}_{m}")
                    for c in range(8):
                        nc.tensor.matmul(
                            ps5[:], w5s[:, c, m * 128:(m + 1) * 128],
                            cat[:, c, :], start=(c == 0), stop=(c == 7))
                    osb = p_osb.tile([128, W], f32, tag="osb",
                                     name=f"osb_{qt0}_{m}")
                    nc.scalar.activation(osb[:], ps5[:], Relu,
                                         bias=sb5s[:, 8 + m:9 + m],
                                         scale=sb5s[:, m:m + 1])
                    nc.sync.dma_start(outr[:, m, qs], osb[:])

            # emission order: conv seg goes out a knn-block late so its
            # input chain overlaps the next knn block; the final segments
            # are 1 qt wide so the post-knn tail chain is short.
            # first block: stage qt 0-2 by key-quarter so the PE has
            # runway while the tail of the input loads.
            with nc.named_scope("knn"):
                for q4 in range(4):
                    for qt in range(3):
                        knn_mm(qt, [2 * q4, 2 * q4 + 1])
                for qt in range(3):
                    knn_top(qt)
                for qt in range(3, 8):
                    knn_qt(qt)
            with nc.named_scope("convs"):
                conv_seg(0, 4)
            with nc.named_scope("knn"):
                for qt in range(8, 12):
                    knn_qt(qt)
            with nc.named_scope("convs"):
                conv_seg(4, 8)
            with nc.named_scope("knn"):
                for qt in range(12, 14):
                    knn_qt(qt)
            with nc.named_scope("convs"):
                conv_seg(8, 12)
            with nc.named_scope("knn"):
                knn_qt(14)
            with nc.named_scope("convs"):
                conv_seg(12, 14)
            with nc.named_scope("knn"):
                knn_qt(15)
            with nc.named_scope("convs"):
                conv_seg(14, 15)
                conv_seg(15, 16)


def prep_inputs(inputs):
    """Host-side sharding + layout/precision prep. Returns per-core in_maps."""
    x = np.ascontiguousarray(inputs["x"], dtype=np.float32)  # [B, C, N]
    shared = {}
    w1 = inputs["w1"].astype(np.float32)
    w1p = np.zeros((CPAD, 128), dtype=np.float16)
    w1p[:C_IN, 0:64] = w1[:, :C_IN].T.astype(np.float16)
    w1p[:C_IN, 64:128] = w1[:, C_IN:].T.astype(np.float16)
    shared["w1t"] = w1p
    shared["w2t"] = np.ascontiguousarray(inputs["w2"].T.astype(np.float16))
    shared["w3t"] = np.ascontiguousarray(inputs["w3"].T.astype(np.float16))
    shared["w4t"] = np.ascontiguousarray(inputs["w4"].T.astype(np.float16))
    w5t = inputs["w5"].astype(np.float32).T  # [960, 1024]
    w5p = np.zeros((128, 8, 1024), dtype=np.float16)
    w5p[0:64, 0, :] = w5t[0:64]          # x1 block
    w5p[:, 1, :] = w5t[64:192]           # x2
    w5p[:, 2, :] = w5t[192:320]          # x3 lo
    w5p[:, 3, :] = w5t[320:448]          # x3 hi
    for m in range(4):                   # x4
        w5p[:, 4 + m, :] = w5t[448 + 128 * m:448 + 128 * (m + 1)]
    shared["w5p"] = w5p

    def scale_bias(i):
        g = inputs[f"g{i}"].astype(np.float32)
        b = inputs[f"b{i}"].astype(np.float32)
        m = inputs[f"m{i}"].astype(np.float32)
        v = inputs[f"v{i}"].astype(np.float32)
        s = g / np.sqrt(v + EPS)
        return s.astype(np.float32), (b - m * s).astype(np.float32)

    s1, b1 = scale_bias(1)
    shared["sb1"] = np.ascontiguousarray(np.stack([s1, b1], axis=1))
    s2, b2 = scale_bias(2)
    shared["sb2"] = np.ascontiguousarray(np.stack([s2, b2], axis=1))
    s3, b3 = scale_bias(3)
    shared["sb3"] = np.ascontiguousarray(
        np.stack([s3[:128], s3[128:], b3[:128], b3[128:]], axis=1))
    s4, b4 = scale_bias(4)
    shared["sb4"] = np.ascontiguousarray(np.stack(
        [s4[128 * m:128 * (m + 1)] for m in range(4)]
        + [b4[128 * m:128 * (m + 1)] for m in range(4)], axis=1))
    s5, b5 = scale_bias(5)
    shared["sb5"] = np.ascontiguousarray(np.stack(
        [s5[128 * m:128 * (m + 1)] for m in range(8)]
        + [b5[128 * m:128 * (m + 1)] for m in range(8)], axis=1))

    in_maps = []
    for core in range(8):
        b, half = core // 2, core % 2
        q0 = half * NQ
        other0 = NQ - q0  # 2048 if half==0 else 0
        xbp = np.concatenate([x[b][:, q0:q0 + NQ], x[b][:, other0:other0 + NQ]],
                             axis=1)  # [1000, 4096], own queries first
        xpad = np.zeros((CPAD, N), dtype=np.float32)
        xpad[:C_IN] = xbp
        xpad[C_IN:C_IN + 3] = 1.0  # query-side ones for the -xx/2 fold
        x16 = xpad.astype(np.float16)
        # key-side chunk 7 with the -xx/2 fold in rows 104-106, built on
        # host: xx from the fp16-quantized points (matches the scores),
        # 3-way fp16 split so the fold is fp32-accurate.
        xk7 = np.zeros((CP, N), dtype=np.float16)
        xk7[0:104] = x16[896:1000]
        r = -0.5 * (x16[:C_IN].astype(np.float32) ** 2).sum(axis=0)
        for j in range(3):
            s = r.astype(np.float16)
            xk7[104 + j] = s
            r = r - s.astype(np.float32)
        m = dict(shared)
        m["xh"] = np.ascontiguousarray(x16)
        m["xk7d"] = xk7
        in_maps.append(m)
    return in_maps


def kernel(**inputs):
    from concourse.bass_utils import run_bass_kernel_spmd

    if "nc" not in _CACHE:
        _CACHE["nc"] = build_nc()
    nc = _CACHE["nc"]
    in_maps = prep_inputs(inputs)
    res = run_bass_kernel_spmd(nc, in_maps, core_ids=list(range(8)))
    out = np.empty((B, 1024, N), dtype=np.float32)
    for core in range(8):
        b, half = core // 2, core % 2
        q0 = half * NQ
        out[b, :, q0:q0 + NQ] = res.results[core]["out"]
    return out


# revision 8
# speedup vs baseline: 1.0165x; 1.0165x over previous
"""DGCNN edge-conv block on 8 Trainium2 NeuronCores.

Sharding: data-parallel over (batch, query-half): core i handles batch i//2,
queries [2048*(i%2) : +2048] of that batch's 4096 points. Each core gets the
full point cloud of its batch (keys) with columns permuted so its own queries
are always columns 0..2047 (SPMD: one program, per-core inputs).

Numerics: x is fp16 everywhere (distances AND conv1 inputs). KNN scores
s/2 = x16_q.x16_k - xx16(k)/2 computed in fp32 PSUM = the exact knn of the
fp16-quantized points; the max-over-k structure makes neighbor ORDER
irrelevant - only the set matters. The -xx/2 per-key offset is computed on
HOST (aux stats, like the BN folding) and rides in spare contraction rows
1000-1002 of a host-built key-side chunk-7 copy (xk7): query side carries
1.0 there, xk7 rows 104-106 carry a 3-way fp16 split of -xx/2, so the whole
score is ONE 8-matmul PSUM group per (query-tile, key-tile).
Conv weights/activations fp16, fp32 PSUM, BN+ReLU writes fp32.

Pipeline per core (conv segments interleave into the knn loop):
  A: load x + xk7 quarter-major; A=w1n@x (duplicated to partitions 64-127
     so the neighbor gather can use all 8 gpsimd cores), Bv=w1c@x_q
  B: per query-tile: 8 key-tiles x 8 fp16 matmuls; top-8 via DVE
     max/max_index; top-3 indices -> ap_gather of A columns -> h1pre;
     h1 = relu(bn1(h1pre+Bv)) applied per query-tile
  C (per query segment, emitted between knn blocks; final segments are
     1 query-tile wide to shrink the tail): conv2..conv4 with max-over-k
     (conv4's BN+ReLU deferred past the max: BN scale > 0), cat,
     conv5 -> out [1024, 2048] fp32
"""

import sys

sys.path.insert(0, "/opt/trn_rl_repo")

import numpy as np

B, C_IN, N, K = 4, 1000, 4096, 3
CPAD = 1024        # padded contraction dim
NQ = 2048          # queries per core
CP = 128           # contraction chunk partitions
CH = 8             # number of contraction chunks
NT = 512           # key tile (psum bank width in fp32)
NNT = N // NT      # 8 key tiles
QT = 128           # query tile (psum partitions)
NQT = NQ // QT     # 16 query tiles
NQH = 1024         # quarter width (phase A / DMA granularity)
EPS = np.float32(1e-5)

_CACHE = {}


def build_nc(finalize=True):
    import concourse.mybir as mybir
    import concourse.tile as tile
    from concourse import bacc

    f32 = mybir.dt.float32
    f16 = mybir.dt.float16
    u16 = mybir.dt.uint16
    i16 = mybir.dt.int16
    Relu = mybir.ActivationFunctionType.Relu

    nc = bacc.Bacc("TRN2", target_bir_lowering=False, debug=False, num_devices=8)

    xh = nc.dram_tensor("xh", [CPAD, N], f16, kind="ExternalInput").ap()
    xk7d = nc.dram_tensor("xk7d", [CP, N], f16, kind="ExternalInput").ap()
    w1t = nc.dram_tensor("w1t", [CPAD, 128], f16, kind="ExternalInput").ap()
    w2t = nc.dram_tensor("w2t", [64, 128], f16, kind="ExternalInput").ap()
    w3t = nc.dram_tensor("w3t", [128, 256], f16, kind="ExternalInput").ap()
    w4t = nc.dram_tensor("w4t", [256, 512], f16, kind="ExternalInput").ap()
    w5p = nc.dram_tensor("w5p", [128, 8, 1024], f16, kind="ExternalInput").ap()
    sb1 = nc.dram_tensor("sb1", [64, 2], f32, kind="ExternalInput").ap()
    sb2 = nc.dram_tensor("sb2", [128, 2], f32, kind="ExternalInput").ap()
    sb3 = nc.dram_tensor("sb3", [128, 4], f32, kind="ExternalInput").ap()
    sb4 = nc.dram_tensor("sb4", [128, 8], f32, kind="ExternalInput").ap()
    sb5 = nc.dram_tensor("sb5", [128, 16], f32, kind="ExternalInput").ap()
    out = nc.dram_tensor("out", [1024, NQ], f32, kind="ExternalOutput").ap()

    with tile.TileContext(nc) as tc:
        _body(nc, tc, mybir, xh, xk7d, w1t, w2t, w3t, w4t, w5p,
              sb1, sb2, sb3, sb4, sb5, out, f32, f16, u16, i16, Relu)
    if finalize:
        nc.finalize()
    return nc


def _body(nc, tc, mybir, xh, xk7d, w1t, w2t, w3t, w4t, w5p,
          sb1, sb2, sb3, sb4, sb5, out, f32, f16, u16, i16, Relu):
    from contextlib import ExitStack
    from concourse import library_config

    es = ExitStack()
    with es:
        p_c1 = es.enter_context(tc.tile_pool(name="c1", bufs=1))

        # gpsimd library for the gathers; dummy gather + drain force the
        # ucode load now so it overlaps the early phases.
        nc.gpsimd.load_library(library_config.ap_gather)
        dmy = p_c1.tile([64, 16], f32, tag="dmy")
        dmys = p_c1.tile([64, 4], f32, tag="dmys")
        dmyi = p_c1.tile([64, 1], i16, tag="dmyi")
        nc.vector.memset(dmys[:], 0.0)
        nc.vector.memset(dmyi[:], 0)
        nc.gpsimd.ap_gather(out_ap=dmy[:], in_ap=dmys[:], idxs_ap=dmyi[:],
                            channels=64, num_elems=4, d=1, num_idxs=16)
        nc.gpsimd.drain()

        # ---- persistent tiles ----
        w1s = p_c1.tile([CP, CH, 128], f16, tag="w1s")
        nc.sync.dma_start(w1s[:], w1t.rearrange("(c p) m -> p c m", p=CP))
        # h1 pre-activation, fp16, kk-major q-ordered [64, 3*NQ]
        h1pre = p_c1.tile([64, 3 * NQ], f16, tag="h1pre")
        # A duplicated on partitions 64-127 so ap_gather uses all 8 cores
        A = p_c1.tile([128, N], f32, tag="A")
        Bv = p_c1.tile([64, NQ], f32, tag="Bv")
        # key-side copy of contraction chunk 7 (built on host): data rows
        # 0-103, rows 104-106 carry the 3-way fp16 split of -xx/2 (query
        # side has 1.0 there), rows 107-127 zero.
        xk7 = p_c1.tile([CP, N], f16, tag="xk7")
        xhs = p_c1.tile([CP, CH, N], f16, tag="xhs")
        xhr = xh.rearrange("(c p) n -> p c n", p=CP)
        # quarter-major loads so phase A / knn can start on quarter 0
        for q4 in range(N // NQH):
            qsl = slice(q4 * NQH, (q4 + 1) * NQH)
            for c in range(CH):
                nc.sync.dma_start(xhs[:, c, qsl], xhr[:, c, qsl])
            nc.sync.dma_start(xk7[:, qsl], xk7d[:, qsl])

        # conv weights + folded BN scale/bias (needed from seg 0 on)
        w2s = p_c1.tile([64, 128], f16, tag="w2s")
        nc.sync.dma_start(w2s[:], w2t[:])
        w3s = p_c1.tile([128, 256], f16, tag="w3s")
        nc.sync.dma_start(w3s[:], w3t[:])
        w4s = p_c1.tile([128, 2, 512], f16, tag="w4s")
        nc.sync.dma_start(w4s[:], w4t.rearrange("(c p) m -> p c m", p=128))
        w5s = p_c1.tile([128, 8, 1024], f16, tag="w5s")
        nc.sync.dma_start(w5s[:], w5p[:])
        sb1s = p_c1.tile([64, 2], f32, tag="sb1s")
        nc.sync.dma_start(sb1s[:], sb1[:])
        sb2s = p_c1.tile([128, 2], f32, tag="sb2s")
        nc.sync.dma_start(sb2s[:], sb2[:])
        sb3s = p_c1.tile([128, 4], f32, tag="sb3s")
        nc.sync.dma_start(sb3s[:], sb3[:])
        sb4s = p_c1.tile([128, 8], f32, tag="sb4s")
        nc.sync.dma_start(sb4s[:], sb4[:])
        sb5s = p_c1.tile([128, 16], f32, tag="sb5s")
        nc.sync.dma_start(sb5s[:], sb5[:])

        # ---- phase A: A/Bv in 4 quarter-passes (xx comes from host) ----
        with nc.named_scope("prep"):
            with tc.tile_pool(name="psa", bufs=2, space="PSUM") as p_psa:
                for q4 in range(N // NQH):
                    nts = [q4 * 2, q4 * 2 + 1]
                    pav = [p_psa.tile([128, NT], f32, tag="pa",
                                      name=f"pa{q4}_{_j}")
                           for _j in range(2)]
                    for c in range(CH):
                        for j, nt in enumerate(nts):
                            mw = 128 if nt < NQ // NT else 64
                            nc.tensor.matmul(
                                pav[j][0:mw, :], w1s[:, c, 0:mw],
                                xhs[:, c, nt * NT:(nt + 1) * NT],
                                start=(c == 0), stop=(c == CH - 1))
                    for j, nt in enumerate(nts):
                        ns = slice(nt * NT, (nt + 1) * NT)
                        nc.scalar.copy(A[0:64, ns], pav[j][0:64, :])
                        nc.scalar.copy(A[64:128, ns], pav[j][0:64, :])
                        if nt < NQ // NT:
                            nc.scalar.copy(Bv[:, ns], pav[j][64:128, :])

        # ---- phase B (knn) with conv segments interleaved ----
        with tc.tile_pool(name="ms", bufs=1) as p_s, \
             tc.tile_pool(name="m8", bufs=3) as p_m8, \
             tc.tile_pool(name="ixw", bufs=4) as p_ixw, \
             tc.tile_pool(name="wtd", bufs=3, space="DRAM") as p_wtd, \
             tc.tile_pool(name="gq", bufs=3) as p_gq, \
             tc.tile_pool(name="seg", bufs=1) as p_seg, \
             tc.tile_pool(name="sg2", bufs=2) as p_sg2, \
             tc.tile_pool(name="tmp", bufs=2) as p_tmp, \
             tc.tile_pool(name="osb", bufs=2) as p_osb, \
             tc.tile_pool(name="pss", bufs=4, space="PSUM") as p_pss, \
             tc.tile_pool(name="psd", bufs=4, space="PSUM") as p_psd:
            outr = out.rearrange("(c p) n -> p c n", p=128)
            h13 = h1pre.rearrange("p (k q) -> p k q", k=3)

            srows = {}

            def knn_mm(qt, nts):
                qs = slice(qt * QT, (qt + 1) * QT)
                if qt not in srows:
                    srows[qt] = p_s.tile([QT, N], f32, tag="srow", bufs=3,
                                         name=f"srow{qt}")
                srow = srows[qt]
                for nt in nts:
                    ns = slice(nt * NT, (nt + 1) * NT)
                    ps = p_pss.tile([QT, NT], f32, tag="pss",
                                    name=f"pss{qt}_{nt}")
                    for c in range(CH - 1):
                        nc.tensor.matmul(ps[:], xhs[:, c, qs],
                                         xhs[:, c, ns],
                                         start=(c == 0), stop=False)
                    nc.tensor.matmul(ps[:], xhs[:, CH - 1, qs], xk7[:, ns],
                                     start=False, stop=True)
                    nc.scalar.copy(srow[:, ns], ps[:])

            def knn_top(qt):
                srow = srows[qt]
                m8 = p_m8.tile([QT, 8], f32, tag="m8")
                i8 = p_m8.tile([QT, 8], u16, tag="i8")
                nc.vector.max(out=m8[:], in_=srow[:])
                nc.vector.max_index(out=i8[:], in_max=m8[:],
                                    in_values=srow[:])
                # wrap top-3 indices into ap_gather layout, one list per
                # query half-tile (gpsimd cores 0-3 serve queries 0-63,
                # cores 4-7 queries 64-127):
                # idxw[16g''+r, 3g+kk] = i8[64h + 16g+r, kk], g''=0..3.
                # i8 bounces through DRAM (where the partition regroup is
                # plain address math) and fans back out on parallel queues.
                idxw = p_ixw.tile([128, 12], i16, tag="idxw")
                wt = p_wtd.tile([128, 3], i16, tag="wt")
                nc.sync.dma_start(wt[:], i8[:, 0:3].bitcast(i16))
                for h in range(2):
                    wr = wt[64 * h:64 * h + 64, :] \
                        .rearrange("(g r) k -> r g k", g=4)
                    for gpp in range(4):
                        p0 = 64 * h + 16 * gpp
                        nc.sync.dma_start(
                            idxw[p0:p0 + 16, :]
                            .rearrange("p (g k) -> p g k", g=4), wr)
                # gather this qt's neighbor features (gpsimd, overlaps PE);
                # within each half, gather position 16*(g*3+kk)+r is
                # (query 16g+r, neighbor kk)
                gq = p_gq.tile([128, 3 * 64], f32, tag="gq")
                nc.gpsimd.ap_gather(
                    out_ap=gq[:], in_ap=A[:], idxs_ap=idxw[:],
                    channels=128, num_elems=N, d=1, num_idxs=3 * 64)
                # unpermute into h1pre (fp16, kk-major q-order)
                gqv = gq.rearrange("p (g kk r) -> p g kk r",
                                   g=4, kk=3, r=16)
                for h in range(2):
                    q0 = qt * QT + 64 * h
                    dst = h13[:, :, q0:q0 + 64] \
                        .rearrange("p kk (g r) -> p g kk r", g=4)
                    nc.scalar.copy(dst, gqv[64 * h:64 * h + 64])
                # h1 = relu(bn1(gathered + center)) in place, fp16
                qs = slice(qt * QT, (qt + 1) * QT)
                hseg = h13[:, :, qs]
                bvb = Bv[:, qs].unsqueeze(1).to_broadcast([64, 3, QT])
                nc.vector.tensor_add(hseg, hseg, bvb)
                nc.scalar.activation(hseg, hseg, Relu,
                                     bias=sb1s[:, 1:2], scale=sb1s[:, 0:1])

            def knn_qt(qt):
                knn_mm(qt, range(NNT))
                knn_top(qt)

            def conv_seg(qt0, qt1):
                qs = slice(qt0 * QT, qt1 * QT)
                W = (qt1 - qt0) * QT
                h2 = p_sg2.tile([128, 3, W], f16, tag="h2", bufs=1,
                                name=f"h2_{qt0}")
                h3 = p_seg.tile([128, 2, 3, W], f16, tag="h3",
                                name=f"h3_{qt0}")
                cat = p_seg.tile([128, 8, W], f16, tag="cat",
                                 name=f"cat_{qt0}")
                nc.vector.memset(cat[64:128, 0, :], 0.0)

                # conv2 (K=64 -> 128)
                for kk in range(3):
                    ps2 = p_psd.tile([128, W], f32, tag="psd",
                                     name=f"ps2_{qt0}_{kk}")
                    nc.tensor.matmul(ps2[:], w2s[:], h13[:, kk, qs],
                                     start=True, stop=True)
                    nc.scalar.activation(h2[:, kk, :], ps2[:], Relu,
                                         bias=sb2s[:, 1:2], scale=sb2s[:, 0:1])
                # x1 -> cat chunk 0 (64 rows)
                t1 = p_tmp.tile([64, W], f16, tag="t64", bufs=1,
                                name=f"t1_{qt0}")
                nc.vector.tensor_max(t1[:], h13[:, 1, qs], h13[:, 2, qs])
                nc.vector.tensor_max(cat[0:64, 0, :], t1[:], h13[:, 0, qs])
                # x2 -> cat chunk 1
                t2 = p_tmp.tile([128, W], f16, tag="t128", name=f"t2_{qt0}")
                nc.vector.tensor_max(t2[:], h2[:, 1, :], h2[:, 2, :])
                nc.vector.tensor_max(cat[:, 1, :], t2[:], h2[:, 0, :])

                # conv3 (K=128 -> 256 in 2 chunks)
                for m in range(2):
                    for kk in range(3):
                        ps3 = p_psd.tile([128, W], f32, tag="psd",
                                         name=f"ps3_{qt0}_{m}_{kk}")
                        nc.tensor.matmul(ps3[:], w3s[:, m * 128:(m + 1) * 128],
                                         h2[:, kk, :], start=True, stop=True)
                        nc.scalar.activation(h3[:, m, kk, :], ps3[:], Relu,
                                             bias=sb3s[:, 2 + m:3 + m],
                                             scale=sb3s[:, m:m + 1])
                # x3 -> cat chunks 2,3
                for m in range(2):
                    t3 = p_tmp.tile([128, W], f16, tag="t128",
                                    name=f"t3_{qt0}_{m}")
                    nc.vector.tensor_max(t3[:], h3[:, m, 1, :], h3[:, m, 2, :])
                    nc.vector.tensor_max(cat[:, 2 + m, :], t3[:], h3[:, m, 0, :])

                # conv4 (K=256 in 2 chunks -> 512 in 4 chunks); BN+ReLU
                # deferred past the max over kk (BN scale > 0)
                for m in range(4):
                    ps4 = [p_psd.tile([128, W], f32, tag="psd",
                                      name=f"ps4_{qt0}_{m}_{kk}")
                           for kk in range(3)]
                    for kk in range(3):
                        for c in range(2):
                            nc.tensor.matmul(
                                ps4[kk][:], w4s[:, c, m * 128:(m + 1) * 128],
                                h3[:, c, kk, :], start=(c == 0), stop=(c == 1))
                    t4 = p_tmp.tile([128, W], f32, tag="t128f", bufs=1,
                                    name=f"t4_{qt0}_{m}")
                    nc.scalar.copy(t4[:], ps4[0][:])
                    nc.vector.tensor_max(t4[:], t4[:], ps4[1][:])
                    nc.vector.tensor_max(t4[:], t4[:], ps4[2][:])
                    nc.scalar.activation(cat[:, 4 + m, :], t4[:], Relu,
                                         bias=sb4s[:, 4 + m:5 + m],
                                         scale=sb4s[:, m:m + 1])

                # conv5 (K=960 padded to 8*128 -> 1024 in 8 chunks)
                for m in range(8):
                    ps5 = p_psd.tile([128, W], f32, tag="psd",
                                     name=f"ps5_{qt0}_{m}")
                    for c in range(8):
                        nc.tensor.matmul(
                            ps5[:], w5s[:, c, m * 128:(m + 1) * 128],
                            cat[:, c, :], start=(c == 0), stop=(c == 7))
                    osb = p_osb.tile([128, W], f32, tag="osb",
                                     name=f"osb_{qt0}_{m}")
                    nc.scalar.activation(osb[:], ps5[:], Relu,
                                         bias=sb5s[:, 8 + m:9 + m],
                                         scale=sb5s[:, m:m + 1])
                    nc.sync.dma_start(outr[:, m, qs], osb[:])

            # emission order: conv seg goes out a knn-block late so its
            # input chain overlaps the next knn block; the final segments
            # are 1 qt wide so the post-knn tail chain is short.
            # first block: stage qt 0-2 by key-quarter so the PE has
            # runway while the tail of the input loads.
            with nc.named_scope("knn"):
                for q4 in range(4):
                    for qt in range(3):
                        knn_mm(qt, [2 * q4, 2 * q4 + 1])
                for qt in range(3):
                    knn_top(qt)
                for qt in range(3, 8):
                    knn_qt(qt)
            with nc.named_scope("convs"):
                conv_seg(0, 4)
            with nc.named_scope("knn"):
                for qt in range(8, 12):
                    knn_qt(qt)
            with nc.named_scope("convs"):
                conv_seg(4, 8)
            with nc.named_scope("knn"):
                for qt in range(12, 14):
                    knn_qt(qt)
            with nc.named_scope("convs"):
                conv_seg(8, 12)
            with nc.named_scope("knn"):
                knn_qt(14)
            with nc.named_scope("convs"):
                conv_seg(12, 14)
            with nc.named_scope("knn"):
                knn_qt(15)
            with nc.named_scope("convs"):
                conv_seg(14, 15)
                conv_seg(15, 16)


def prep_inputs(inputs):
    """Host-side sharding + layout/precision prep. Returns per-core in_maps."""
    x = np.ascontiguousarray(inputs["x"], dtype=np.float32)  # [B, C, N]
    shared = {}
    w1 = inputs["w1"].astype(np.float32)
    w1p = np.zeros((CPAD, 128), dtype=np.float16)
    w1p[:C_IN, 0:64] = w1[:, :C_IN].T.astype(np.float16)
    w1p[:C_IN, 64:128] = w1[:, C_IN:].T.astype(np.float16)
    shared["w1t"] = w1p
    shared["w2t"] = np.ascontiguousarray(inputs["w2"].T.astype(np.float16))
    shared["w3t"] = np.ascontiguousarray(inputs["w3"].T.astype(np.float16))
    shared["w4t"] = np.ascontiguousarray(inputs["w4"].T.astype(np.float16))
    w5t = inputs["w5"].astype(np.float32).T  # [960, 1024]
    w5p = np.zeros((128, 8, 1024), dtype=np.float16)
    w5p[0:64, 0, :] = w5t[0:64]          # x1 block
    w5p[:, 1, :] = w5t[64:192]           # x2
    w5p[:, 2, :] = w5t[192:320]          # x3 lo
    w5p[:, 3, :] = w5t[320:448]          # x3 hi
    for m in range(4):                   # x4
        w5p[:, 4 + m, :] = w5t[448 + 128 * m:448 + 128 * (m + 1)]
    shared["w5p"] = w5p

    def scale_bias(i):
        g = inputs[f"g{i}"].astype(np.float32)
        b = inputs[f"b{i}"].astype(np.float32)
        m = inputs[f"m{i}"].astype(np.float32)
        v = inputs[f"v{i}"].astype(np.float32)
        s = g / np.sqrt(v + EPS)
        return s.astype(np.float32), (b - m * s).astype(np.float32)

    s1, b1 = scale_bias(1)
    shared["sb1"] = np.ascontiguousarray(np.stack([s1, b1], axis=1))
    s2, b2 = scale_bias(2)
    shared["sb2"] = np.ascontiguousarray(np.stack([s2, b2], axis=1))
    s3, b3 = scale_bias(3)
    shared["sb3"] = np.ascontiguousarray(
        np.stack([s3[:128], s3[128:], b3[:128], b3[128:]], axis=1))
    s4, b4 = scale_bias(4)
    shared["sb4"] = np.ascontiguousarray(np.stack(
        [s4[128 * m:128 * (m + 1)] for m in range(4)]
        + [b4[128 * m:128 * (m + 1)] for m in range(4)], axis=1))
    s5, b5 = scale_bias(5)
    shared["sb5"] = np.ascontiguousarray(np.stack(
        [s5[128 * m:128 * (m + 1)] for m in range(8)]
        + [b5[128 * m:128 * (m + 1)] for m in range(8)], axis=1))

    in_maps = []
    for core in range(8):
        b, half = core // 2, core % 2
        q0 = half * NQ
        other0 = NQ - q0  # 2048 if half==0 else 0
        xbp = np.concatenate([x[b][:, q0:q0 + NQ], x[b][:, other0:other0 + NQ]],
                             axis=1)  # [1000, 4096], own queries first
        xpad = np.zeros((CPAD, N), dtype=np.float32)
        xpad[:C_IN] = xbp
        xpad[C_IN:C_IN + 3] = 1.0  # query-side ones for the -xx/2 fold
        x16 = xpad.astype(np.float16)
        # key-side chunk 7 with the -xx/2 fold in rows 104-106, built on
        # host: xx from the fp16-quantized points (matches the scores),
        # 3-way fp16 split so the fold is fp32-accurate.
        xk7 = np.zeros((CP, N), dtype=np.float16)
        xk7[0:104] = x16[896:1000]
        r = -0.5 * (x16[:C_IN].astype(np.float32) ** 2).sum(axis=0)
        for j in range(3):
            s = r.astype(np.float16)
            xk7[104 + j] = s
            r = r - s.astype(np.float32)
        m = dict(shared)
        m["xh"] = np.ascontiguousarray(x16)
        m["xk7d"] = xk7
        in_maps.append(m)
    return in_maps


def kernel(**inputs):
    from concourse.bass_utils import run_bass_kernel_spmd

    if "nc" not in _CACHE:
        _CACHE["nc"] = build_nc()
    nc = _CACHE["nc"]
    in_maps = prep_inputs(inputs)
    res = run_bass_kernel_spmd(nc, in_maps, core_ids=list(range(8)))
    out = np.empty((B, 1024, N), dtype=np.float32)
    for core in range(8):
        b, half = core // 2, core % 2
        q0 = half * NQ
        out[b, :, q0:q0 + NQ] = res.results[core]["out"]
    return out


# revision 10
# speedup vs baseline: 1.1031x; 1.0853x over previous
"""DGCNN edge-conv block on 8 Trainium2 NeuronCores.

Sharding: data-parallel over (batch, query-half): core i handles batch i//2,
queries [2048*(i%2) : +2048] of that batch's 4096 points. Each core gets the
full point cloud of its batch (keys) with columns permuted so its own queries
are always columns 0..2047 (SPMD: one program, per-core inputs).

Numerics: x is fp16 everywhere (distances AND conv1 inputs). KNN scores
s/2 = x16_q.x16_k - xx16(k)/2 computed in fp32 PSUM = the exact knn of the
fp16-quantized points; the max-over-k structure makes neighbor ORDER
irrelevant - only the set matters. The -xx/2 per-key offset is computed on
HOST (aux stats, like the BN folding) and rides in spare contraction rows
1000-1002 of a host-built key-side chunk-7 copy (xk7): query side carries
1.0 there, xk7 rows 104-106 carry a 3-way fp16 split of -xx/2, so the whole
score is ONE 8-matmul PSUM group per (query-tile, key-tile).
Conv weights/activations fp16, fp32 PSUM, BN+ReLU writes fp32.

Pipeline per core (conv segments interleave into the knn loop):
  A: load x + xk7 quarter-major; A=w1n@x (duplicated to partitions 64-127
     so the neighbor gather can use all 8 gpsimd cores), Bv=w1c@x_q
  B: per query-tile: 8 key-tiles x 8 fp16 matmuls; top-8 via DVE
     max/max_index; top-3 indices -> ap_gather of A columns -> h1pre;
     h1 = relu(bn1(h1pre+Bv)) applied per query-tile
  C (per query segment, emitted between knn blocks; final segments are
     1 query-tile wide to shrink the tail): conv2..conv4 with max-over-k
     (conv4's BN+ReLU deferred past the max: BN scale > 0), cat,
     conv5 -> out [1024, 2048] fp32
"""

import sys

sys.path.insert(0, "/opt/trn_rl_repo")

import numpy as np

B, C_IN, N, K = 4, 1000, 4096, 3
CPAD = 1024        # padded contraction dim
NQ = 2048          # queries per core
CP = 128           # contraction chunk partitions
CH = 8             # number of contraction chunks
NT = 512           # key tile (psum bank width in fp32)
NNT = N // NT      # 8 key tiles
QT = 128           # query tile (psum partitions)
NQT = NQ // QT     # 16 query tiles
NQH = 1024         # quarter width (phase A / DMA granularity)
EPS = np.float32(1e-5)

_CACHE = {}


def build_nc(finalize=True):
    import concourse.mybir as mybir
    import concourse.tile as tile
    from concourse import bacc

    f32 = mybir.dt.float32
    f16 = mybir.dt.float16
    u16 = mybir.dt.uint16
    i16 = mybir.dt.int16
    Relu = mybir.ActivationFunctionType.Relu

    nc = bacc.Bacc("TRN2", target_bir_lowering=False, debug=False, num_devices=8)

    xh = nc.dram_tensor("xh", [CPAD, N], f16, kind="ExternalInput").ap()
    xk7d = nc.dram_tensor("xk7d", [CP, N], f16, kind="ExternalInput").ap()
    w1t = nc.dram_tensor("w1t", [CPAD, 128], f16, kind="ExternalInput").ap()
    w2t = nc.dram_tensor("w2t", [64, 128], f16, kind="ExternalInput").ap()
    w3t = nc.dram_tensor("w3t", [128, 256], f16, kind="ExternalInput").ap()
    w4t = nc.dram_tensor("w4t", [256, 512], f16, kind="ExternalInput").ap()
    w5p = nc.dram_tensor("w5p", [128, 8, 1024], f16, kind="ExternalInput").ap()
    sb1 = nc.dram_tensor("sb1", [64, 2], f32, kind="ExternalInput").ap()
    sb2 = nc.dram_tensor("sb2", [128, 2], f32, kind="ExternalInput").ap()
    sb3 = nc.dram_tensor("sb3", [128, 4], f32, kind="ExternalInput").ap()
    sb4 = nc.dram_tensor("sb4", [128, 8], f32, kind="ExternalInput").ap()
    sb5 = nc.dram_tensor("sb5", [128, 16], f32, kind="ExternalInput").ap()
    out = nc.dram_tensor("out", [1024, NQ], f32, kind="ExternalOutput").ap()

    with tile.TileContext(nc) as tc:
        _body(nc, tc, mybir, xh, xk7d, w1t, w2t, w3t, w4t, w5p,
              sb1, sb2, sb3, sb4, sb5, out, f32, f16, u16, i16, Relu)
    if finalize:
        nc.finalize()
    return nc


def _body(nc, tc, mybir, xh, xk7d, w1t, w2t, w3t, w4t, w5p,
          sb1, sb2, sb3, sb4, sb5, out, f32, f16, u16, i16, Relu):
    from contextlib import ExitStack
    from concourse import library_config

    es = ExitStack()
    with es:
        p_c1 = es.enter_context(tc.tile_pool(name="c1", bufs=1))

        # gpsimd library for the gathers; dummy gather + drain force the
        # ucode load now so it overlaps the early phases.
        nc.gpsimd.load_library(library_config.ap_gather)
        dmy = p_c1.tile([64, 16], f32, tag="dmy")
        dmys = p_c1.tile([64, 4], f32, tag="dmys")
        dmyi = p_c1.tile([64, 1], i16, tag="dmyi")
        nc.vector.memset(dmys[:], 0.0)
        nc.vector.memset(dmyi[:], 0)
        nc.gpsimd.ap_gather(out_ap=dmy[:], in_ap=dmys[:], idxs_ap=dmyi[:],
                            channels=64, num_elems=4, d=1, num_idxs=16)
        nc.gpsimd.drain()

        # ---- persistent tiles ----
        w1s = p_c1.tile([CP, CH, 128], f16, tag="w1s")
        nc.sync.dma_start(w1s[:], w1t.rearrange("(c p) m -> p c m", p=CP))
        # h1 pre-activation, fp16, kk-major q-ordered [64, 3*NQ]
        h1pre = p_c1.tile([64, 3 * NQ], f16, tag="h1pre")
        # A duplicated on partitions 64-127 so ap_gather uses all 8 cores
        A = p_c1.tile([128, N], f32, tag="A")
        Bv = p_c1.tile([64, NQ], f32, tag="Bv")
        # key-side copy of contraction chunk 7 (built on host): data rows
        # 0-103, rows 104-106 carry the 3-way fp16 split of -xx/2 (query
        # side has 1.0 there), rows 107-127 zero.
        xk7 = p_c1.tile([CP, N], f16, tag="xk7")
        xhs = p_c1.tile([CP, CH, N], f16, tag="xhs")
        xhr = xh.rearrange("(c p) n -> p c n", p=CP)
        # quarter-major loads so phase A / knn can start on quarter 0
        for q4 in range(N // NQH):
            qsl = slice(q4 * NQH, (q4 + 1) * NQH)
            for c in range(CH):
                nc.sync.dma_start(xhs[:, c, qsl], xhr[:, c, qsl])
            nc.sync.dma_start(xk7[:, qsl], xk7d[:, qsl])

        # conv weights + folded BN scale/bias (needed from seg 0 on)
        w2s = p_c1.tile([64, 128], f16, tag="w2s")
        nc.sync.dma_start(w2s[:], w2t[:])
        w3s = p_c1.tile([128, 256], f16, tag="w3s")
        nc.sync.dma_start(w3s[:], w3t[:])
        w4s = p_c1.tile([128, 2, 512], f16, tag="w4s")
        nc.sync.dma_start(w4s[:], w4t.rearrange("(c p) m -> p c m", p=128))
        w5s = p_c1.tile([128, 8, 1024], f16, tag="w5s")
        nc.sync.dma_start(w5s[:], w5p[:])
        sb1s = p_c1.tile([64, 2], f32, tag="sb1s")
        nc.sync.dma_start(sb1s[:], sb1[:])
        sb2s = p_c1.tile([128, 2], f32, tag="sb2s")
        nc.sync.dma_start(sb2s[:], sb2[:])
        sb3s = p_c1.tile([128, 4], f32, tag="sb3s")
        nc.sync.dma_start(sb3s[:], sb3[:])
        sb4s = p_c1.tile([128, 8], f32, tag="sb4s")
        nc.sync.dma_start(sb4s[:], sb4[:])
        sb5s = p_c1.tile([128, 16], f32, tag="sb5s")
        nc.sync.dma_start(sb5s[:], sb5[:])

        # ---- phase A: A/Bv in 4 quarter-passes (xx comes from host) ----
        with nc.named_scope("prep"):
            with tc.tile_pool(name="psa", bufs=2, space="PSUM") as p_psa:
                for q4 in range(N // NQH):
                    nts = [q4 * 2, q4 * 2 + 1]
                    pav = [p_psa.tile([128, NT], f32, tag="pa",
                                      name=f"pa{q4}_{_j}")
                           for _j in range(2)]
                    for c in range(CH):
                        for j, nt in enumerate(nts):
                            mw = 128 if nt < NQ // NT else 64
                            nc.tensor.matmul(
                                pav[j][0:mw, :], w1s[:, c, 0:mw],
                                xhs[:, c, nt * NT:(nt + 1) * NT],
                                start=(c == 0), stop=(c == CH - 1))
                    for j, nt in enumerate(nts):
                        ns = slice(nt * NT, (nt + 1) * NT)
                        nc.scalar.copy(A[0:64, ns], pav[j][0:64, :])
                        nc.scalar.copy(A[64:128, ns], pav[j][0:64, :])
                        if nt < NQ // NT:
                            nc.scalar.copy(Bv[:, ns], pav[j][64:128, :])

        # ---- phase B (knn) with conv segments interleaved ----
        with tc.tile_pool(name="ms", bufs=1) as p_s, \
             tc.tile_pool(name="m8", bufs=3) as p_m8, \
             tc.tile_pool(name="ixw", bufs=4) as p_ixw, \
             tc.tile_pool(name="gq", bufs=3) as p_gq, \
             tc.tile_pool(name="seg", bufs=1) as p_seg, \
             tc.tile_pool(name="sg2", bufs=2) as p_sg2, \
             tc.tile_pool(name="tmp", bufs=2) as p_tmp, \
             tc.tile_pool(name="osb", bufs=2) as p_osb, \
             tc.tile_pool(name="pss", bufs=4, space="PSUM") as p_pss, \
             tc.tile_pool(name="psd", bufs=4, space="PSUM") as p_psd:
            outr = out.rearrange("(c p) n -> p c n", p=128)
            h13 = h1pre.rearrange("p (k q) -> p k q", k=3)

            srows = {}

            def knn_mm(qt, nts):
                qs = slice(qt * QT, (qt + 1) * QT)
                if qt not in srows:
                    srows[qt] = p_s.tile([QT, N], f32, tag="srow", bufs=3,
                                         name=f"srow{qt}")
                srow = srows[qt]
                for nt in nts:
                    ns = slice(nt * NT, (nt + 1) * NT)
                    ps = p_pss.tile([QT, NT], f32, tag="pss",
                                    name=f"pss{qt}_{nt}")
                    for c in range(CH - 1):
                        nc.tensor.matmul(ps[:], xhs[:, c, qs],
                                         xhs[:, c, ns],
                                         start=(c == 0), stop=False)
                    nc.tensor.matmul(ps[:], xhs[:, CH - 1, qs], xk7[:, ns],
                                     start=False, stop=True)
                    nc.scalar.copy(srow[:, ns], ps[:])

            def knn_top(qt):
                srow = srows[qt]
                m8 = p_m8.tile([QT, 8], f32, tag="m8")
                i8 = p_m8.tile([QT, 8], u16, tag="i8")
                nc.vector.max(out=m8[:], in_=srow[:])
                nc.vector.max_index(out=i8[:], in_max=m8[:],
                                    in_values=srow[:])
                # wrap top-3 indices into ap_gather layout, one list per
                # query half-tile (gpsimd cores 0-3 serve queries 0-63,
                # cores 4-7 queries 64-127):
                # idxw[16g''+r, 3g+kk] = i8[64h + 16g+r, kk], g''=0..3
                idxw = p_ixw.tile([128, 12], i16, tag="idxw")
                for h in range(2):
                    p0 = 64 * h
                    for g in range(4):
                        nc.sync.dma_start(
                            idxw[p0:p0 + 16, 3 * g:3 * g + 3],
                            i8[p0 + 16 * g:p0 + 16 * g + 16, 0:3]
                            .bitcast(i16))
                    nc.sync.dma_start(idxw[p0 + 16:p0 + 32, :],
                                      idxw[p0:p0 + 16, :])
                    nc.sync.dma_start(idxw[p0 + 32:p0 + 64, :],
                                      idxw[p0:p0 + 32, :])
                # gather this qt's neighbor features (gpsimd, overlaps PE);
                # within each half, gather position 16*(g*3+kk)+r is
                # (query 16g+r, neighbor kk)
                gq = p_gq.tile([128, 3 * 64], f32, tag="gq")
                nc.gpsimd.ap_gather(
                    out_ap=gq[:], in_ap=A[:], idxs_ap=idxw[:],
                    channels=128, num_elems=N, d=1, num_idxs=3 * 64)
                # unpermute into h1pre (fp16, kk-major q-order)
                gqv = gq.rearrange("p (g kk r) -> p g kk r",
                                   g=4, kk=3, r=16)
                for h in range(2):
                    q0 = qt * QT + 64 * h
                    dst = h13[:, :, q0:q0 + 64] \
                        .rearrange("p kk (g r) -> p g kk r", g=4)
                    nc.scalar.copy(dst, gqv[64 * h:64 * h + 64])
                # h1 = relu(bn1(gathered + center)) in place, fp16
                qs = slice(qt * QT, (qt + 1) * QT)
                hseg = h13[:, :, qs]
                bvb = Bv[:, qs].unsqueeze(1).to_broadcast([64, 3, QT])
                nc.vector.tensor_add(hseg, hseg, bvb)
                nc.scalar.activation(hseg, hseg, Relu,
                                     bias=sb1s[:, 1:2], scale=sb1s[:, 0:1])

            def knn_qt(qt):
                knn_mm(qt, range(NNT))
                knn_top(qt)

            def conv_seg(qt0, qt1):
                qs = slice(qt0 * QT, qt1 * QT)
                W = (qt1 - qt0) * QT
                h2 = p_sg2.tile([128, 3, W], f16, tag="h2", bufs=1,
                                name=f"h2_{qt0}")
                h3 = p_seg.tile([128, 2, 3, W], f16, tag="h3",
                                name=f"h3_{qt0}")
                cat = p_seg.tile([128, 8, W], f16, tag="cat",
                                 name=f"cat_{qt0}")
                nc.vector.memset(cat[64:128, 0, :], 0.0)

                # conv2 (K=64 -> 128)
                for kk in range(3):
                    ps2 = p_psd.tile([128, W], f32, tag="psd",
                                     name=f"ps2_{qt0}_{kk}")
                    nc.tensor.matmul(ps2[:], w2s[:], h13[:, kk, qs],
                                     start=True, stop=True)
                    nc.scalar.activation(h2[:, kk, :], ps2[:], Relu,
                                         bias=sb2s[:, 1:2], scale=sb2s[:, 0:1])
                # x1 -> cat chunk 0 (64 rows)
                t1 = p_tmp.tile([64, W], f16, tag="t64", bufs=1,
                                name=f"t1_{qt0}")
                nc.vector.tensor_max(t1[:], h13[:, 1, qs], h13[:, 2, qs])
                nc.vector.tensor_max(cat[0:64, 0, :], t1[:], h13[:, 0, qs])
                # x2 -> cat chunk 1
                t2 = p_tmp.tile([128, W], f16, tag="t128", name=f"t2_{qt0}")
                nc.vector.tensor_max(t2[:], h2[:, 1, :], h2[:, 2, :])
                nc.vector.tensor_max(cat[:, 1, :], t2[:], h2[:, 0, :])

                # conv3 (K=128 -> 256 in 2 chunks)
                for m in range(2):
                    for kk in range(3):
                        ps3 = p_psd.tile([128, W], f32, tag="psd",
                                         name=f"ps3_{qt0}_{m}_{kk}")
                        nc.tensor.matmul(ps3[:], w3s[:, m * 128:(m + 1) * 128],
                                         h2[:, kk, :], start=True, stop=True)
                        nc.scalar.activation(h3[:, m, kk, :], ps3[:], Relu,
                                             bias=sb3s[:, 2 + m:3 + m],
                                             scale=sb3s[:, m:m + 1])
                # x3 -> cat chunks 2,3
                for m in range(2):
                    t3 = p_tmp.tile([128, W], f16, tag="t128",
                                    name=f"t3_{qt0}_{m}")
                    nc.vector.tensor_max(t3[:], h3[:, m, 1, :], h3[:, m, 2, :])
                    nc.vector.tensor_max(cat[:, 2 + m, :], t3[:], h3[:, m, 0, :])

                # conv4 (K=256 in 2 chunks -> 512 in 4 chunks); BN+ReLU
                # deferred past the max over kk (BN scale > 0)
                for m in range(4):
                    ps4 = [p_psd.tile([128, W], f32, tag="psd",
                                      name=f"ps4_{qt0}_{m}_{kk}")
                           for kk in range(3)]
                    for kk in range(3):
                        for c in range(2):
                            nc.tensor.matmul(
                                ps4[kk][:], w4s[:, c, m * 128:(m + 1) * 128],
                                h3[:, c, kk, :], start=(c == 0), stop=(c == 1))
                    t4 = p_tmp.tile([128, W], f32, tag="t128f", bufs=1,
                                    name=f"t4_{qt0}_{m}")
                    nc.scalar.copy(t4[:], ps4[0][:])
                    nc.vector.tensor_max(t4[:], t4[:], ps4[1][:])
                    nc.vector.tensor_max(t4[:], t4[:], ps4[2][:])
                    nc.scalar.activation(cat[:, 4 + m, :], t4[:], Relu,
                                         bias=sb4s[:, 4 + m:5 + m],
                                         scale=sb4s[:, m:m + 1])

                # conv5 (K=960 padded to 8*128 -> 1024 in 8 chunks)
                for m in range(8):
                    ps5 = p_psd.tile([128, W], f32, tag="psd",
                                     name=f"ps5_{qt0}_{m}")
                    for c in range(8):
                        nc.tensor.matmul(
                            ps5[:], w5s[:, c, m * 128:(m + 1) * 128],
                            cat[:, c, :], start=(c == 0), stop=(c == 7))
                    osb = p_osb.tile([128, W], f32, tag="osb",
                                     name=f"osb_{qt0}_{m}")
                    nc.scalar.activation(osb[:], ps5[:], Relu,
                                         bias=sb5s[:, 8 + m:9 + m],
                                         scale=sb5s[:, m:m + 1])
                    nc.sync.dma_start(outr[:, m, qs], osb[:])

            # emission order: conv seg goes out a knn-block late so its
            # input chain overlaps the next knn block; the final segments
            # are 1 qt wide so the post-knn tail chain is short.
            # first block: stage qt 0-2 by key-quarter so the PE has
            # runway while the tail of the input loads.
            with nc.named_scope("knn"):
                for q4 in range(4):
                    for qt in range(3):
                        knn_mm(qt, [2 * q4, 2 * q4 + 1])
                for qt in range(3):
                    knn_top(qt)
                for qt in range(3, 8):
                    knn_qt(qt)
            with nc.named_scope("convs"):
                conv_seg(0, 4)
            with nc.named_scope("knn"):
                for qt in range(8, 12):
                    knn_qt(qt)
            with nc.named_scope("convs"):
                conv_seg(4, 8)
            with nc.named_scope("knn"):
                for qt in range(12, 14):
                    knn_qt(qt)
            with nc.named_scope("convs"):
                conv_seg(8, 12)
            with nc.named_scope("knn"):
                knn_qt(14)
            with nc.named_scope("convs"):
                conv_seg(12, 14)
            with nc.named_scope("knn"):
                knn_qt(15)
            with nc.named_scope("convs"):
                conv_seg(14, 15)
                conv_seg(15, 16)


def prep_inputs(inputs):
    """Host-side sharding + layout/precision prep. Returns per-core in_maps."""
    x = np.ascontiguousarray(inputs["x"], dtype=np.float32)  # [B, C, N]
    shared = {}
    w1 = inputs["w1"].astype(np.float32)
    w1p = np.zeros((CPAD, 128), dtype=np.float16)
    w1p[:C_IN, 0:64] = w1[:, :C_IN].T.astype(np.float16)
    w1p[:C_IN, 64:128] = w1[:, C_IN:].T.astype(np.float16)
    shared["w1t"] = w1p
    shared["w2t"] = np.ascontiguousarray(inputs["w2"].T.astype(np.float16))
    shared["w3t"] = np.ascontiguousarray(inputs["w3"].T.astype(np.float16))
    shared["w4t"] = np.ascontiguousarray(inputs["w4"].T.astype(np.float16))
    w5t = inputs["w5"].astype(np.float32).T  # [960, 1024]
    w5p = np.zeros((128, 8, 1024), dtype=np.float16)
    w5p[0:64, 0, :] = w5t[0:64]          # x1 block
    w5p[:, 1, :] = w5t[64:192]           # x2
    w5p[:, 2, :] = w5t[192:320]          # x3 lo
    w5p[:, 3, :] = w5t[320:448]          # x3 hi
    for m in range(4):                   # x4
        w5p[:, 4 + m, :] = w5t[448 + 128 * m:448 + 128 * (m + 1)]
    shared["w5p"] = w5p

    def scale_bias(i):
        g = inputs[f"g{i}"].astype(np.float32)
        b = inputs[f"b{i}"].astype(np.float32)
        m = inputs[f"m{i}"].astype(np.float32)
        v = inputs[f"v{i}"].astype(np.float32)
        s = g / np.sqrt(v + EPS)
        return s.astype(np.float32), (b - m * s).astype(np.float32)

    s1, b1 = scale_bias(1)
    shared["sb1"] = np.ascontiguousarray(np.stack([s1, b1], axis=1))
    s2, b2 = scale_bias(2)
    shared["sb2"] = np.ascontiguousarray(np.stack([s2, b2], axis=1))
    s3, b3 = scale_bias(3)
    shared["sb3"] = np.ascontiguousarray(
        np.stack([s3[:128], s3[128:], b3[:128], b3[128:]], axis=1))
    s4, b4 = scale_bias(4)
    shared["sb4"] = np.ascontiguousarray(np.stack(
        [s4[128 * m:128 * (m + 1)] for m in range(4)]
        + [b4[128 * m:128 * (m + 1)] for m in range(4)], axis=1))
    s5, b5 = scale_bias(5)
    shared["sb5"] = np.ascontiguousarray(np.stack(
        [s5[128 * m:128 * (m + 1)] for m in range(8)]
        + [b5[128 * m:128 * (m + 1)] for m in range(8)], axis=1))

    in_maps = []
    for core in range(8):
        b, half = core // 2, core % 2
        q0 = half * NQ
        other0 = NQ - q0  # 2048 if half==0 else 0
        xbp = np.concatenate([x[b][:, q0:q0 + NQ], x[b][:, other0:other0 + NQ]],
                             axis=1)  # [1000, 4096], own queries first
        xpad = np.zeros((CPAD, N), dtype=np.float32)
        xpad[:C_IN] = xbp
        xpad[C_IN:C_IN + 3] = 1.0  # query-side ones for the -xx/2 fold
        x16 = xpad.astype(np.float16)
        # key-side chunk 7 with the -xx/2 fold in rows 104-106, built on
        # host: xx from the fp16-quantized points (matches the scores),
        # 3-way fp16 split so the fold is fp32-accurate.
        xk7 = np.zeros((CP, N), dtype=np.float16)
        xk7[0:104] = x16[896:1000]
        r = -0.5 * (x16[:C_IN].astype(np.float32) ** 2).sum(axis=0)
        for j in range(3):
            s = r.astype(np.float16)
            xk7[104 + j] = s
            r = r - s.astype(np.float32)
        m = dict(shared)
        m["xh"] = np.ascontiguousarray(x16)
        m["xk7d"] = xk7
        in_maps.append(m)
    return in_maps


def kernel(**inputs):
    from concourse.bass_utils import run_bass_kernel_spmd

    if "nc" not in _CACHE:
        _CACHE["nc"] = build_nc()
    nc = _CACHE["nc"]
    in_maps = prep_inputs(inputs)
    res = run_bass_kernel_spmd(nc, in_maps, core_ids=list(range(8)))
    out = np.empty((B, 1024, N), dtype=np.float32)
    for core in range(8):
        b, half = core // 2, core % 2
        q0 = half * NQ
        out[b, :, q0:q0 + NQ] = res.results[core]["out"]
    return out
